# revision 1
# baseline (speedup 1.0000x reference)
"""Trainium2 Bass kernel for nn_DiagScanModule: anti-diagonal scan reorder.

For each (b, c) plane of x (8, 64, 512, 512), produce two length-262144
sequences: the plane's elements in 'rd' anti-diagonal order (d = i+j,
i ascending within a diagonal) and 'ld' order (d = j-i+511, i ascending).

Strategy (per core; batch-sharded across 8 cores):
  The elements of rd-diagonal d live at flat offsets 511*i + d (stride 511);
  ld-diagonal d at 513*i + d - 511.  Define the sheared matrix
  W[i, d] = x_flat[stride*i + d + doff]: column d of W is diagonal d.
  1. HWDGE (sync queue) shear-loads W chunks [128, 32, 128] f32 — 512B
     descriptors; HBM reads are latency-hidden so small read descriptors
     are cheap.  Always full 128 partitions / 128 columns: odd partition
     counts or odd element sizes fall off the DMA fast path (~15x slower);
     a generous back-pad keeps the over-read in bounds.
  2. PE-transposes 128xR f32 tiles -> PSUM; DVE copies into
     V[d_partition, channel, i_slot] in SBUF.
  3. One output DMA per (diagonal, 32-channel group): 32 descriptors of
     ln[d]*4 bytes each to the exact y offsets.  Stores round-robin over
     the scalar + gpsimd DMA queues only (sync stays dedicated to loads:
     a queue is FIFO, and a 2MiB load occupies it for ~10us, stalling any
     store queued behind it).  Within each 128-diagonal block, stores are
     issued in a stride-8 partition order so concurrent DMAs source
     different SBUF AXI ports / SDMA engines (HBM write descriptors are
     ~175ns latency-bound each; port-serial order leaves engines idle).
The index maps are compile-time constants of H=W=512 (reference's
_diag_maps), so all offsets/lengths are hardcoded into the access patterns
and the index-map inputs are not read on device.
"""

import os

import numpy as np

import concourse.bass as bass
import concourse.mybir as mybir
from concourse import masks
from concourse.tile import TileContext
from concourse.bass_utils import run_bass_kernel_spmd

# ---------------------------------------------------------------- geometry

B, C, H, W = 8, 64, 512, 512
HW = H * W            # 262144
ND = H + W - 1        # 1023 diagonals
PAD = 512             # front pad (elements) so ld's earliest reads stay in-bounds
BACKPAD = 70000       # tail pad: loads always read full [128,CG,128] tiles;
                      # the over-read reaches up to ~stride*127+1023 elements
                      # past the plane end
XLEN = PAD + C * HW + BACKPAD
CG = 32               # channels per group (per output DMA)
DBLK = 128            # diagonals per block (= PE transpose width)

F32 = mybir.dt.float32

STORE_ENG = "ag"      # scalar + gpsimd queues for stores
LOAD_ENG = "s"        # sync queue for loads


def _geom(kind):
    d = np.arange(ND)
    ln = 512 - np.abs(511 - d)
    if kind == "rd":
        stride, doff = 511, 0
        s = np.maximum(0, d - 511)
    else:
        stride, doff = 513, -511
        s = np.maximum(0, 511 - d)
    off = np.concatenate([[0], np.cumsum(ln)[:-1]])
    return stride, doff, s.astype(int), ln.astype(int), off.astype(int)


# ---------------------------------------------------------------- tile patch

def _patch_tile_drain():
    """walrus in this container rejects the TileContext exit drain when it
    carries semaphore waits ('Too many sync wait commands').  Emit the waits
    as individual NoOps instead and keep drains waitless."""
    import concourse.tile as tile_mod
    from concourse.vector_clock import ScopedClock

    if getattr(tile_mod.TileContext, "_diag_drain_patched", False):
        return

    def _drain_and_barrier(self, tick_clock, wait_clock):
        nc = self.nc
        drain_inst = nc.sync.drain(fusable=False)
        wait_clock.add_sem_waits(
            drain_inst.ins, ScopedClock({None: tick_clock.global_clock})
        )
        si = drain_inst.ins.sync_info
        waits = list(si.on_wait) if si is not None else []
        if waits:
            drain_inst.ins.sync_info = mybir.SyncInfo(on_wait=[], on_update=[])
            for w in waits:
                ni = nc.sync.nop()
                ni.ins.sync_info = mybir.SyncInfo(on_wait=[w], on_update=[])
            nc.sync.drain(fusable=False)

        nc.all_engine_barrier()
        assert self.sems is not None
        popped = nc._tile_sem_poison_stack.pop()
        assert popped is self._sem_poison
        nc.clear_and_free_semaphores(list(self.sems.allocated().values()))
        nc.all_engine_barrier()

    tile_mod.TileContext._drain_and_barrier = _drain_and_barrier
    tile_mod.TileContext._diag_drain_patched = True


def _split_multi_waits(nc, max_waits=1):
    """walrus here rejects instructions carrying more than one semaphore
    wait ('Too many sync wait commands').  Hoist excess waits onto NoOps
    inserted just before the instruction on the same engine — the engine
    blocks on each in program order, which preserves the sync semantics."""
    k = 0
    for fn in nc.m.functions:
        for bb in fn.blocks:
            new = []
            dirty = False
            for inst in bb.instructions:
                si = inst.sync_info
                waits = list(si.on_wait) if si is not None else []
                if len(waits) > max_waits:
                    for w in waits[:-max_waits]:
                        nop = mybir.InstNoOp(name=f"WSPLIT-{k}", ins=[], outs=[])
                        k += 1
                        nop.engine = inst.engine
                        nop.sync_info = mybir.SyncInfo(on_wait=[w], on_update=[])
                        new.append(nop)
                    inst.sync_info = mybir.SyncInfo(
                        on_wait=waits[-max_waits:], on_update=list(si.on_update)
                    )
                    dirty = True
                new.append(inst)
            if dirty:
                bb.instructions = new


# ---------------------------------------------------------------- kernel build

def _build_nc():
    _patch_tile_drain()
    nc = bass.Bass()
    x_t = nc.dram_tensor("x", [XLEN], F32, kind="ExternalInput")
    y_t = {
        "rd": nc.dram_tensor("y_rd", [C * HW], F32, kind="ExternalOutput"),
        "ld": nc.dram_tensor("y_ld", [C * HW], F32, kind="ExternalOutput"),
    }

    with TileContext(nc) as tc:
        with (
            tc.tile_pool(name="const", bufs=1) as cpool,
            tc.tile_pool(name="w", bufs=2) as wpool,
            tc.tile_pool(name="v", bufs=2) as vpool,
            tc.tile_pool(name="ps", bufs=8, space="PSUM") as ppool,
        ):
            ident = cpool.tile([128, 128], F32, tag="ident")
            masks.make_identity(nc, ident[:])

            emap = {"s": nc.sync, "a": nc.scalar, "g": nc.gpsimd}
            store_engs = [emap[ch] for ch in STORE_ENG]
            load_engs = [emap[ch] for ch in LOAD_ENG]
            se = le = 0

            geom = {k: _geom(k) for k in ("rd", "ld")}
            nblk = (ND + DBLK - 1) // DBLK
            half = nblk // 2
            border = [b for p in range(half) for b in (p, half + p)] + (
                [nblk - 1] if nblk % 2 else []
            )
            # pair rd block b with ld block nblk-1-b: they read the same x
            # rows (HBM read locality); interleave light/heavy blocks so
            # SDMA descriptor demand stays even across the kernel
            sched = []
            for cg in range(C // CG):
                for b in border:
                    sched.append(("rd", cg, b))
                    sched.append(("ld", cg, nblk - 1 - b))
            for kind, cg, blk in sched:
                stride, doff, s, ln, off = geom[kind]
                if True:
                    cbase = cg * CG
                    for d0 in [blk * DBLK]:
                        Dd = min(DBLK, ND - d0)
                        ds = np.arange(d0, d0 + Dd)
                        i_lo = int(np.min(s[ds]))
                        i_hi = int(np.max(s[ds] + ln[ds]))
                        span = i_hi - i_lo
                        nchunk = (span + 127) // 128

                        V = vpool.tile([128, CG, 512], F32, tag="V")
                        for k in range(nchunk):
                            r0 = i_lo + 128 * k
                            R = min(128, i_hi - r0)
                            Wt = wpool.tile([128, CG, DBLK], F32, tag="W")
                            src = bass.AP(
                                x_t,
                                PAD + cbase * HW + stride * r0 + d0 + doff,
                                [[stride, 128], [HW, CG], [1, DBLK]],
                            )
                            leng = load_engs[le % len(load_engs)]
                            le += 1
                            leng.dma_start(out=Wt[:, :, :], in_=src)
                            for c in range(CG):
                                P = ppool.tile([128, 128], F32, tag="P")
                                nc.tensor.transpose(
                                    P[:Dd, :R], Wt[:R, c, :Dd], ident[:R, :R]
                                )
                                nc.vector.tensor_copy(
                                    V[:Dd, c, 128 * k : 128 * k + R], P[:Dd, :R]
                                )

                        # port-spread store order: stride 8 over partitions
                        for dd in [x for r in range(8) for x in range(r, Dd, 8)]:
                            d = d0 + dd
                            a = int(s[d]) - i_lo
                            L = int(ln[d])
                            dst = bass.AP(
                                y_t[kind],
                                cbase * HW + int(off[d]),
                                [[HW, CG], [1, L]],
                            )
                            seng = store_engs[se % len(store_engs)]
                            se += 1
                            seng.dma_start(out=dst, in_=V[dd : dd + 1, :, a : a + L])
    _split_multi_waits(nc)
    return nc


_NC_CACHE = None
LAST_RESULTS = None


def kernel(x, rd_index_map=None, ld_index_map=None):
    """Full-input entry point: x (8, 64, 512, 512) f32 -> (y_rd, y_ld),
    each (8, 64, 262144) f32.  Index maps are deterministic functions of
    H=W=512 (see reference _diag_maps) and are baked into the kernel's
    access patterns, so they are not read here."""
    global _NC_CACHE, LAST_RESULTS
    x = np.ascontiguousarray(np.asarray(x), dtype=np.float32)
    assert x.shape == (B, C, H, W), x.shape

    if _NC_CACHE is None:
        _NC_CACHE = _build_nc()
    nc = _NC_CACHE

    in_maps = []
    for b in range(B):
        xb = np.zeros(XLEN, np.float32)
        xb[PAD:PAD + C * HW] = x[b].reshape(-1)
        in_maps.append({"x": xb})

    trace = bool(int(os.environ.get("DIAG_TRACE", "0")))
    res = run_bass_kernel_spmd(
        nc,
        in_maps,
        core_ids=list(range(B)),
        trace=trace,
    )
    LAST_RESULTS = res

    y_rd = np.empty((B, C, HW), np.float32)
    y_ld = np.empty((B, C, HW), np.float32)
    for b in range(B):
        y_rd[b] = res.results[b]["y_rd"].reshape(C, HW)
        y_ld[b] = res.results[b]["y_ld"].reshape(C, HW)
    return (y_rd, y_ld)

